# revision 9
# baseline (speedup 1.0000x reference)
"""Trainium2 Bass kernel for nn_CrowdsClassificationSModel.

Reference computation:
    W = softmax(kernel, axis=1)            # (8, 8, 59)
    out = einsum('bc,cdr->bdr', x, W)      # (131072, 8, 59)
    out = where(drop_mask, out / 0.6, 0)

Strategy (data-parallel over 8 NeuronCores, batch-sharded):
  - softmax of the tiny (8,8,59) kernel is computed on host, scaled by
    1/keep, flattened to W (8, 472) and zero-padded into a (32, 4*472)
    block-layout so that K=32 matmuls select one of 4 batch sub-tiles.
  - per core (16384 batches): x is loaded as [128, 1024] (partition p
    holds batches p*128..p*128+127), PE-transposed 32 columns at a time
    into [32, 128] lhsT tiles (rows = (n_local, c) pairs), then one
    K=32, M=128, N=472 float32r matmul per 128-batch tile.
  - dropout: DVE multiplies the PSUM matmul result by the uint8 keep
    mask (0/1) into an SBUF staging tile, which is DMA'd out in
    4-batch-row chunks (1888 f32 per partition line).
"""

import numpy as np

import concourse.bacc as bacc
import concourse.bass as bass
import concourse.tile as tile
from concourse import mybir
from concourse.bass_utils import run_bass_kernel_spmd

N_CORES = 8
B_FULL = 131072
C = 8
R = 59
F = C * R  # 472
DROP_RATE = 0.4
KEEP = np.float32(1.0 - DROP_RATE)
NT = 4  # batch sub-tiles per supertile (n-values per partition line)


def softmax_np(k: np.ndarray, axis: int) -> np.ndarray:
    k = k.astype(np.float64)
    m = k.max(axis=axis, keepdims=True)
    e = np.exp(k - m)
    return (e / e.sum(axis=axis, keepdims=True)).astype(np.float32)


def build_w32(kernel: np.ndarray) -> np.ndarray:
    """(8,8,59) raw kernel -> (32, NT*472) f32 zero-padded, keep-scaled."""
    w = softmax_np(kernel, axis=1).reshape(C, F) / KEEP  # (8, 472)
    w32 = np.zeros((4 * C, NT * F), dtype=np.float32)
    for k in range(NT):
        w32[8 * k : 8 * (k + 1), k * F : (k + 1) * F] = w
    return w32


def build_module(bc: int) -> tuple[bass.Bass, dict]:
    """Build the per-core Bass module for a batch shard of size bc."""
    assert bc % (128 * NT) == 0
    n_total = bc // 128  # batches per partition
    n_super = n_total // NT  # supertiles
    fs = NT * F  # free elems per supertile line

    nc = bacc.Bacc("TRN2", target_bir_lowering=False, debug=False)
    f32 = mybir.dt.float32
    f32r = mybir.dt.float32r
    u8 = mybir.dt.uint8

    x_d = nc.dram_tensor("x_sh", (bc, C), f32r, kind="ExternalInput")
    m_d = nc.dram_tensor("mask_sh", (bc, F), u8, kind="ExternalInput")
    w_d = nc.dram_tensor("w32", (4 * C, fs), f32r, kind="ExternalInput")
    o_d = nc.dram_tensor("out_sh", (bc, F), f32, kind="ExternalOutput")
    id_d = nc.dram_tensor("ident128", (128, 128), f32r, kind="ExternalInput")

    # b = p * n_total + s*NT + k
    m_view = m_d[:].rearrange("(p s k) f -> s p (k f)", p=128, s=n_super, k=NT)
    o_view = o_d[:].rearrange("(p s k) f -> s p (k f)", p=128, s=n_super, k=NT)
    x_view = x_d[:].rearrange("(p n) c -> p (n c)", p=128)

    with tile.TileContext(nc) as tc:
        with (
            tc.tile_pool(name="const", bufs=1) as constp,
            tc.tile_pool(name="xl", bufs=1) as xlp,
            tc.tile_pool(name="xt", bufs=3) as xtp,
            tc.tile_pool(name="mask", bufs=4) as maskp,
            tc.tile_pool(name="stage", bufs=4) as stagep,
            tc.tile_pool(name="pt", bufs=2, space="PSUM") as ptp,
            tc.tile_pool(name="pmm", bufs=6, space="PSUM") as pmmp,
        ):
            ident = constp.tile([128, 128], f32r)
            nc.sync.dma_start(ident[:], id_d[:])
            w32 = constp.tile([4 * C, fs], f32r)
            nc.sync.dma_start(w32[:], w_d[:])
            x_l = xlp.tile([128, n_total * C], f32r)
            nc.sync.dma_start(x_l[:], x_view)

            for s in range(n_super):
                # transpose 32 columns of x_l -> lhsT for NT batch tiles
                pt = ptp.tile([4 * C, 128], f32r)
                nc.tensor.transpose(
                    pt[:], x_l[:, s * 4 * C : (s + 1) * 4 * C], ident[:]
                )
                xt = xtp.tile([4 * C, 128], f32r)
                nc.scalar.copy(xt[:], pt[:])

                mt = maskp.tile([128, fs], u8)
                nc.scalar.dma_start(mt[:], m_view[s])

                st = stagep.tile([128, fs], f32)
                for k in range(NT):
                    pm = pmmp.tile([128, F], f32)
                    nc.tensor.matmul(
                        pm[:],
                        xt[:],
                        w32[:, k * F : (k + 1) * F],
                        start=True,
                        stop=True,
                    )
                    nc.vector.tensor_mul(
                        st[:, k * F : (k + 1) * F], pm[:], mt[:, k * F : (k + 1) * F]
                    )

                nc.sync.dma_start(o_view[s], st[:])

    nc.compile()
    names = {"x": "x_sh", "mask": "mask_sh", "w": "w32", "out": "out_sh"}
    return nc, names


_CACHE: dict = {}


def _get_module(bc: int):
    if bc not in _CACHE:
        _CACHE[bc] = build_module(bc)
    return _CACHE[bc]


def _prep_inputs(x, kernel, drop_mask, bc):
    w32 = build_w32(np.asarray(kernel))
    x = np.ascontiguousarray(np.asarray(x, dtype=np.float32))
    mask = np.asarray(drop_mask)
    if mask.dtype != np.uint8:
        mask = mask.astype(np.uint8)
    mask = np.ascontiguousarray(mask.reshape(mask.shape[0], -1))
    n_shards = x.shape[0] // bc
    in_maps = []
    for i in range(n_shards):
        in_maps.append(
            {
                "x_sh": x[i * bc : (i + 1) * bc],
                "mask_sh": mask[i * bc : (i + 1) * bc],
                "w32": w32,
                "ident128": np.eye(128, dtype=np.float32),
            }
        )
    return in_maps


def run(x, kernel, drop_mask, trace: bool = False):
    bc = x.shape[0] // N_CORES
    nc, names = _get_module(bc)
    in_maps = _prep_inputs(x, kernel, drop_mask, bc)
    res = run_bass_kernel_spmd(
        nc, in_maps, core_ids=list(range(N_CORES)), trace=trace
    )
    out = np.concatenate([r[names["out"]] for r in res.results], axis=0)
    return out.reshape(B_FULL, C, R), res


def kernel(x, kernel, drop_mask) -> np.ndarray:
    out, _ = run(x, kernel, drop_mask, trace=False)
    return out
